# revision 5
# baseline (speedup 1.0000x reference)
"""DTW layer kernel for Trainium2 (8 NeuronCores, SPMD data-parallel).

Problem: for each (batch b, filter f) pair, run the DTW dynamic program
    D[i,j] = (x[b,i]-k[f,j])^2 + min(D[i-1,j], D[i,j-1], D[i-1,j-1])
over an N x M grid and emit D[i, M-1] for all i.  B=256, F=64, N=2048, M=16.

Sharding: batch is split 32-per-core across 8 cores (every (b,f) DP is
independent); kernels are replicated.

Per-core layout: 128 partitions = 4 filter-groups x 32 local batches; the
free dim packs the 16 filters of the group as 16 blocks of 17 (1 "spacer"
slot + 16 j-cells).  Per DP row i, three instructions do all the work for
all 2048 (b,f) problems on the core:
  - ScalarE activation: d = Square(-K + x[:,i])   (local cost row)
  - DVE tensor_tensor:  A = min(D_prev, D_prev shifted by 1)  (up vs diag)
  - DVE tensor_tensor_scan: D_cur = (A min state) + d  along the free dim
    (the j-recurrence; spacer slots carry a huge cost so the scan state is
    reset between adjacent filter blocks)
A fourth ScalarE copy extracts D[i, M-1] per filter into an output chunk
that is DMA'd to DRAM every CHUNK rows.
"""

import sys

if "/opt/trn_rl_repo" not in sys.path:
    sys.path.insert(0, "/opt/trn_rl_repo")

import numpy as np

B, F, N, M = 256, 64, 2048, 16
NCORES = 8
BLOC = B // NCORES          # 32 batches per core
NFG = 4                     # filter groups (of 16) per core
S = M + 1                   # 17: spacer + 16 j-cells
FD = 16 * S                 # 272 free elements per DP row
NDBUF = 8                   # rotating D-row buffers
CHUNK = 256                 # rows per output chunk
BIG = 1.0e30                # +inf stand-in for DP boundaries
KSPACER = -1.0e18           # kernel value at spacer slots -> d ~ 1e36

_cached = {}


def _patch_tile_tail_drain():
    """This walrus build rejects >2 sync waits on one instruction; Tile's
    tail drain attaches one wait per outstanding proc.  Split them into
    one SP nop per proc."""
    import concourse.tile as tile_mod
    from concourse.vector_clock import ScopedClock, VectorClock

    def _patched(self, tick_clock, wait_clock):
        g = tick_clock.global_clock
        n = len(g)
        for proc in range(n):
            t = g[proc]
            if t > 0:
                vec = [0] * n
                vec[proc] = t
                nop = self.nc.sync.nop()
                wait_clock.add_sem_waits(
                    nop.ins, ScopedClock({None: VectorClock(vec)})
                )
        self.nc.sync.drain()
        self.nc.all_engine_barrier()
        assert self.sems is not None
        popped = self.nc._tile_sem_poison_stack.pop()
        assert popped is self._sem_poison
        self.nc.clear_and_free_semaphores(list(self.sems.allocated().values()))
        self.nc.all_engine_barrier()

    tile_mod.TileContext._drain_and_barrier = _patched


def _build(n_rows=N, chunk=CHUNK):
    import concourse.bacc as bacc_mod
    import concourse.bass as bass
    import concourse.mybir as mybir
    from concourse.tile import TileContext

    _patch_tile_tail_drain()

    f32 = mybir.dt.float32
    AFT = mybir.ActivationFunctionType
    OP = mybir.AluOpType

    # Bacc (not raw Bass): its finalize() runs generate_event_semaphores,
    # which splits multi-sem waits into EVSEM insts (walrus caps waits/inst).
    nc = bacc_mod.Bacc()
    xs = nc.declare_dram_parameter("x", [BLOC, N], f32, isOutput=False)
    ks = nc.declare_dram_parameter("kernels", [F, M], f32, isOutput=False)
    out = nc.declare_dram_parameter("out", [BLOC, F, n_rows], f32, isOutput=True)

    with TileContext(nc) as tc:
        with (
            tc.tile_pool(name="consts", bufs=1) as consts,
            tc.tile_pool(name="dpool", bufs=6) as dpool,
            tc.tile_pool(name="apool", bufs=4) as apool,
            tc.tile_pool(name="opool", bufs=2) as opool,
        ):
            x_sb = consts.tile([128, N], f32)
            K_sb = consts.tile([128, FD], f32)
            Kstg = consts.tile([128, F // NFG * M], f32)
            Dbufs = consts.tile([128, NDBUF * (FD + 1)], f32)

            # x into group 0's partitions, then partition-shifted copies to
            # replicate across the other filter groups
            nc.gpsimd.dma_start(out=x_sb[0:32, :], in_=xs[:, :])
            for fg in range(1, NFG):
                nc.gpsimd.dma_start(
                    out=x_sb[fg * 32 : (fg + 1) * 32, :], in_=x_sb[0:32, :]
                )

            # K layout: spacer slots = KSPACER, j slots = kernels[fg*16+p, j],
            # identical across the 32 batch partitions of each group.  Stage
            # the group's contiguous 16x16 block per partition via a
            # partition-broadcast DMA, then spread into the spacered layout
            # with a strided on-chip copy.
            nc.vector.memset(K_sb[:], KSPACER)
            Kview = K_sb.rearrange("q (p s) -> q p s", s=S)
            for fg in range(NFG):
                ksl = ks[fg * 16 : (fg + 1) * 16, :]
                src = bass.AP(
                    tensor=ksl.tensor,
                    offset=ksl.offset,
                    ap=[[0, 32], [1, 16 * M]],
                )
                nc.gpsimd.dma_start(out=Kstg[fg * 32 : (fg + 1) * 32, :], in_=src)
            nc.vector.tensor_copy(
                out=Kview[:, :, 1:],
                in_=Kstg.rearrange("q (p j) -> q p j", j=M),
            )

            # D buffers: all BIG; virtual row D[-1] (slot NDBUF-1) gets 0 in
            # its spacer slots so cell (0,0) sees diag 0 while (0,j>0) sees inf
            nc.vector.memset(Dbufs[:], BIG)
            dinit = Dbufs[:, (NDBUF - 1) * (FD + 1) : NDBUF * (FD + 1)]
            dinit_sp = dinit[:, 1:].rearrange("q (p s) -> q p s", s=S)[:, :, 0:1]
            nc.vector.memset(dinit_sp, 0.0)

            Dsl = [Dbufs[:, r * (FD + 1) : (r + 1) * (FD + 1)] for r in range(NDBUF)]

            for c in range(n_rows // chunk):
                och = opool.tile([128, 16 * chunk], f32)
                ochv = och.rearrange("q (p t) -> q p t", t=chunk)
                for t in range(chunk):
                    i = c * chunk + t
                    Dprev = Dsl[(i - 1) % NDBUF]
                    Dcur = Dsl[i % NDBUF]
                    d_t = dpool.tile([128, FD], f32)
                    nc.scalar.activation(
                        out=d_t[:],
                        in_=K_sb[:],
                        func=AFT.Square,
                        bias=x_sb[:, i : i + 1],
                        scale=-1.0,
                    )
                    a_t = apool.tile([128, FD], f32)
                    nc.vector.tensor_tensor(
                        out=a_t[:],
                        in0=Dprev[:, 1 : FD + 1],
                        in1=Dprev[:, 0:FD],
                        op=OP.min,
                    )
                    nc.vector.tensor_tensor_scan(
                        out=Dcur[:, 1 : FD + 1],
                        data0=a_t[:],
                        data1=d_t[:],
                        initial=BIG,
                        op0=OP.min,
                        op1=OP.add,
                    )
                    nc.scalar.copy(
                        out=ochv[:, :, t : t + 1],
                        in_=Dcur[:, 1:].rearrange("q (p s) -> q p s", s=S)[:, :, M : M + 1],
                    )
                for fg in range(NFG):
                    nc.sync.dma_start(
                        out=out[:, fg * 16 : (fg + 1) * 16, c * chunk : (c + 1) * chunk],
                        in_=ochv[fg * 32 : (fg + 1) * 32, :, :],
                    )
    # run_bass_via_pjrt serializes the module without finalizing; Bacc's
    # register allocation + EVSEM wait-splitting happen in finalize().
    nc.finalize()
    return nc


def _get_nc():
    if "nc" not in _cached:
        _cached["nc"] = _build()
    return _cached["nc"]


def kernel(x, kernels):
    from concourse.bass_utils import run_bass_kernel_spmd

    x = np.asarray(x, dtype=np.float32)
    kernels = np.asarray(kernels, dtype=np.float32)
    nc = _get_nc()
    in_maps = [
        {"x": x[c * BLOC : (c + 1) * BLOC], "kernels": kernels}
        for c in range(NCORES)
    ]
    res = run_bass_kernel_spmd(nc, in_maps, core_ids=list(range(NCORES)))
    return np.concatenate([res.results[c]["out"] for c in range(NCORES)], axis=0)


# revision 6
# speedup vs baseline: 1.0105x; 1.0105x over previous
"""DTW layer kernel for Trainium2 (8 NeuronCores, SPMD data-parallel).

Problem: for each (batch b, filter f) pair, run the DTW dynamic program
    D[i,j] = (x[b,i]-k[f,j])^2 + min(D[i-1,j], D[i,j-1], D[i-1,j-1])
over an N x M grid and emit D[i, M-1] for all i.  B=256, F=64, N=2048, M=16.

Sharding: batch is split 32-per-core across 8 cores (every (b,f) DP is
independent); kernels are replicated.

Per-core layout: 128 partitions = 4 filter-groups x 32 local batches; the
free dim packs the 16 filters of the group as 16 blocks of 17 (1 "spacer"
slot + 16 j-cells).  Per DP row i, three instructions do all the work for
all 2048 (b,f) problems on the core:
  - ScalarE activation: d = Square(-K + x[:,i])   (local cost row)
  - DVE tensor_tensor:  A = min(D_prev, D_prev shifted by 1)  (up vs diag)
  - DVE tensor_tensor_scan: D_cur = (A min state) + d  along the free dim
    (the j-recurrence; spacer slots carry a huge cost so the scan state is
    reset between adjacent filter blocks)
A fourth ScalarE copy extracts D[i, M-1] per filter into an output chunk
that is DMA'd to DRAM every CHUNK rows.
"""

import sys

if "/opt/trn_rl_repo" not in sys.path:
    sys.path.insert(0, "/opt/trn_rl_repo")

import numpy as np

B, F, N, M = 256, 64, 2048, 16
NCORES = 8
BLOC = B // NCORES          # 32 batches per core
NFG = 4                     # filter groups (of 16) per core
S = M + 1                   # 17: spacer + 16 j-cells
FD = 16 * S                 # 272 free elements per DP row
NDBUF = 16                  # rotating D-row buffers
CHUNK = 256                 # rows per output chunk
BIG = 1.0e30                # +inf stand-in for DP boundaries
KSPACER = -1.0e18           # kernel value at spacer slots -> d ~ 1e36

_cached = {}


def _patch_tile_tail_drain():
    """This walrus build rejects >2 sync waits on one instruction; Tile's
    tail drain attaches one wait per outstanding proc.  Split them into
    one SP nop per proc."""
    import concourse.tile as tile_mod
    from concourse.vector_clock import ScopedClock, VectorClock

    def _patched(self, tick_clock, wait_clock):
        g = tick_clock.global_clock
        n = len(g)
        for proc in range(n):
            t = g[proc]
            if t > 0:
                vec = [0] * n
                vec[proc] = t
                nop = self.nc.sync.nop()
                wait_clock.add_sem_waits(
                    nop.ins, ScopedClock({None: VectorClock(vec)})
                )
        self.nc.sync.drain()
        self.nc.all_engine_barrier()
        assert self.sems is not None
        popped = self.nc._tile_sem_poison_stack.pop()
        assert popped is self._sem_poison
        self.nc.clear_and_free_semaphores(list(self.sems.allocated().values()))
        self.nc.all_engine_barrier()

    tile_mod.TileContext._drain_and_barrier = _patched


def _build(n_rows=N, chunk=CHUNK):
    import concourse.bacc as bacc_mod
    import concourse.bass as bass
    import concourse.mybir as mybir
    from concourse.tile import TileContext

    _patch_tile_tail_drain()

    f32 = mybir.dt.float32
    AFT = mybir.ActivationFunctionType
    OP = mybir.AluOpType

    # Bacc (not raw Bass): its finalize() runs generate_event_semaphores,
    # which splits multi-sem waits into EVSEM insts (walrus caps waits/inst).
    nc = bacc_mod.Bacc()
    xs = nc.declare_dram_parameter("x", [BLOC, N], f32, isOutput=False)
    ks = nc.declare_dram_parameter("kernels", [F, M], f32, isOutput=False)
    out = nc.declare_dram_parameter("out", [BLOC, F, n_rows], f32, isOutput=True)

    with TileContext(nc) as tc:
        with (
            tc.tile_pool(name="consts", bufs=1) as consts,
            tc.tile_pool(name="dpool", bufs=8) as dpool,
            tc.tile_pool(name="apool", bufs=6) as apool,
            tc.tile_pool(name="opool", bufs=2) as opool,
        ):
            x_sb = consts.tile([128, N], f32)
            K_sb = consts.tile([128, FD], f32)
            Kstg = consts.tile([128, F // NFG * M], f32)
            Dbufs = consts.tile([128, NDBUF * (FD + 1)], f32)

            # x into group 0's partitions, then partition-shifted copies to
            # replicate across the other filter groups
            nc.gpsimd.dma_start(out=x_sb[0:32, :], in_=xs[:, :])
            for fg in range(1, NFG):
                nc.gpsimd.dma_start(
                    out=x_sb[fg * 32 : (fg + 1) * 32, :], in_=x_sb[0:32, :]
                )

            # K layout: spacer slots = KSPACER, j slots = kernels[fg*16+p, j],
            # identical across the 32 batch partitions of each group.  Stage
            # the group's contiguous 16x16 block per partition via a
            # partition-broadcast DMA, then spread into the spacered layout
            # with a strided on-chip copy.
            nc.vector.memset(K_sb[:], KSPACER)
            Kview = K_sb.rearrange("q (p s) -> q p s", s=S)
            for fg in range(NFG):
                ksl = ks[fg * 16 : (fg + 1) * 16, :]
                src = bass.AP(
                    tensor=ksl.tensor,
                    offset=ksl.offset,
                    ap=[[0, 32], [1, 16 * M]],
                )
                nc.gpsimd.dma_start(out=Kstg[fg * 32 : (fg + 1) * 32, :], in_=src)
            nc.vector.tensor_copy(
                out=Kview[:, :, 1:],
                in_=Kstg.rearrange("q (p j) -> q p j", j=M),
            )

            # D buffers: all BIG; virtual row D[-1] (slot NDBUF-1) gets 0 in
            # its spacer slots so cell (0,0) sees diag 0 while (0,j>0) sees inf
            nc.vector.memset(Dbufs[:], BIG)
            dinit = Dbufs[:, (NDBUF - 1) * (FD + 1) : NDBUF * (FD + 1)]
            dinit_sp = dinit[:, 1:].rearrange("q (p s) -> q p s", s=S)[:, :, 0:1]
            nc.vector.memset(dinit_sp, 0.0)

            Dsl = [Dbufs[:, r * (FD + 1) : (r + 1) * (FD + 1)] for r in range(NDBUF)]

            for c in range(n_rows // chunk):
                och = opool.tile([128, 16 * chunk], f32)
                ochv = och.rearrange("q (p t) -> q p t", t=chunk)
                for t in range(chunk):
                    i = c * chunk + t
                    Dprev = Dsl[(i - 1) % NDBUF]
                    Dcur = Dsl[i % NDBUF]
                    d_t = dpool.tile([128, FD], f32)
                    nc.scalar.activation(
                        out=d_t[:],
                        in_=K_sb[:],
                        func=AFT.Square,
                        bias=x_sb[:, i : i + 1],
                        scale=-1.0,
                    )
                    a_t = apool.tile([128, FD], f32)
                    nc.vector.tensor_tensor(
                        out=a_t[:],
                        in0=Dprev[:, 1 : FD + 1],
                        in1=Dprev[:, 0:FD],
                        op=OP.min,
                    )
                    nc.vector.tensor_tensor_scan(
                        out=Dcur[:, 1 : FD + 1],
                        data0=a_t[:],
                        data1=d_t[:],
                        initial=BIG,
                        op0=OP.min,
                        op1=OP.add,
                    )
                    nc.scalar.copy(
                        out=ochv[:, :, t : t + 1],
                        in_=Dcur[:, 1:].rearrange("q (p s) -> q p s", s=S)[:, :, M : M + 1],
                    )
                for fg in range(NFG):
                    nc.sync.dma_start(
                        out=out[:, fg * 16 : (fg + 1) * 16, c * chunk : (c + 1) * chunk],
                        in_=ochv[fg * 32 : (fg + 1) * 32, :, :],
                    )
    # run_bass_via_pjrt serializes the module without finalizing; Bacc's
    # register allocation + EVSEM wait-splitting happen in finalize().
    nc.finalize()
    return nc


def _get_nc():
    if "nc" not in _cached:
        _cached["nc"] = _build()
    return _cached["nc"]


def kernel(x, kernels):
    from concourse.bass_utils import run_bass_kernel_spmd

    x = np.asarray(x, dtype=np.float32)
    kernels = np.asarray(kernels, dtype=np.float32)
    nc = _get_nc()
    in_maps = [
        {"x": x[c * BLOC : (c + 1) * BLOC], "kernels": kernels}
        for c in range(NCORES)
    ]
    res = run_bass_kernel_spmd(nc, in_maps, core_ids=list(range(NCORES)))
    return np.concatenate([res.results[c]["out"] for c in range(NCORES)], axis=0)
